# revision 1
# baseline (speedup 1.0000x reference)
"""Trainium2 Bass kernel for CommunityPreservationLoss (triplet margin loss
over pairwise distances with hardest-negative mining).

Strategy (8 NeuronCores, SPMD):
  - Shard anchor rows: 1024 rows/core = 8 blocks of 128 anchors.
  - Per block, the PE assembles d2 = sq_i + sq_j - 2*x_i.x_j directly in PSUM
    with bf16 matmuls: the fp32 operand -2*X is split into bf16 hi+lo, and
    x.x' is computed as hi@hi + hi@lo + lo@hi (the lo@lo term is ~2^-18 and
    dropped); sq_j is added via a K=3 ones @ [sq_hi;sq_mid;sq_lo] matmul and
    sq_i rides the per-partition bias of the sqrt evacuation.
  - ACT evacuates PSUM with dist = Sqrt(psum + sq_i).
  - DVE tensor_scalar builds maskshift = -64 * (comm_j != comm_i)  (bf16).
  - DVE tensor_tensor adds it in place: md = dist + maskshift. Different-
    community pairs sit at dist-64 (all negative), same-community pairs at
    dist (>0), so row-min(md) + 64 = hardest-negative distance.
  - tensor_reduce(min) extracts the row min (VectorE or GpSimd, see
    REDUCE_ENGINE).
  - ACT computes Relu(md + (margin - minneg)) with a fused row-sum
    (accum_out): same-community pairs contribute the triplet margin, the
    diagonal and different-community pairs relu to zero.
  - Host sums the 8x[128,8] partials (f64) and divides by the exact triplet
    count from a bincount of communities.

The diagonal is handled by inflating the anchor-side sq by 2e-3: d2_ii stays
positive (no sqrt NaN), dist_ii ~ 0.05 which is excluded from the min (diff
pairs are < 0) and from the pos-sum (0.05 - c < 0 for this data regime).
"""

import numpy as np
import ml_dtypes

BF16 = ml_dtypes.bfloat16

N = 8192          # nodes
D = 128           # embedding dim
NCORES = 8
RPC = N // NCORES  # rows per core = 1024
NBLK = RPC // 128  # anchor blocks per core = 8
GRP = 2048         # psum tile width (4 banks)
SUB = 512          # matmul moving width
MARGIN = 1.0
SHIFT = 64.0       # additive mask shift for different-community pairs
DIAG_EPS = 2e-3    # anchor-side sq inflation (keeps diagonal d2 > 0)

REDUCE_ENGINE = "vector"   # free-dim reduce is VectorE-only

_cache = {}


def _build_nc():
    import concourse.tile as tile
    from concourse import bacc, mybir

    f32 = mybir.dt.float32
    bf16 = mybir.dt.bfloat16
    AF = mybir.ActivationFunctionType
    OP = mybir.AluOpType

    nc = bacc.Bacc("TRN2", target_bir_lowering=False, debug=False)

    xth_d = nc.dram_tensor("xth", [D, N], bf16, kind="ExternalInput").ap()
    xtl_d = nc.dram_tensor("xtl", [D, N], bf16, kind="ExternalInput").ap()
    m2h_d = nc.dram_tensor("m2h", [D, RPC], bf16, kind="ExternalInput").ap()
    m2l_d = nc.dram_tensor("m2l", [D, RPC], bf16, kind="ExternalInput").ap()
    sqr_d = nc.dram_tensor("sqr", [3, N], bf16, kind="ExternalInput").ap()
    one_d = nc.dram_tensor("one", [3, D], bf16, kind="ExternalInput").ap()
    sqa_d = nc.dram_tensor("sqa", [128, NBLK], f32, kind="ExternalInput").ap()
    cmb_d = nc.dram_tensor("cmb", [128, N], bf16, kind="ExternalInput").ap()
    cma_d = nc.dram_tensor("cma", [128, NBLK], f32, kind="ExternalInput").ap()
    out_d = nc.dram_tensor("possum", [128, NBLK], f32, kind="ExternalOutput").ap()

    with tile.TileContext(nc) as tc:
        with (
            tc.tile_pool(name="const", bufs=1) as constp,
            tc.tile_pool(name="dist", bufs=2) as distp,
            tc.tile_pool(name="msk", bufs=2) as mskp,
            tc.tile_pool(name="small", bufs=4) as smallp,
            tc.tile_pool(name="ps", bufs=2, space="PSUM") as psp,
        ):
            xth_s = constp.tile([D, N], bf16, tag="xth")
            xtl_s = constp.tile([D, N], bf16, tag="xtl")
            m2h_s = constp.tile([D, RPC], bf16, tag="m2h")
            m2l_s = constp.tile([D, RPC], bf16, tag="m2l")
            sqr_s = constp.tile([3, N], bf16, tag="sqr")
            one_s = constp.tile([3, D], bf16, tag="one")
            sqa_s = constp.tile([128, NBLK], f32, tag="sqa")
            cmb_s = constp.tile([128, N], bf16, tag="cmb")
            cma_s = constp.tile([128, NBLK], f32, tag="cma")
            possum_s = constp.tile([128, NBLK], f32, tag="possum")

            # small operands first so block-0 matmuls can start ASAP,
            # then xt chunks in compute order, then the mask operands
            nc.sync.dma_start(out=m2h_s[:], in_=m2h_d[:])
            nc.sync.dma_start(out=m2l_s[:], in_=m2l_d[:])
            nc.sync.dma_start(out=sqr_s[:], in_=sqr_d[:])
            nc.sync.dma_start(out=one_s[:], in_=one_d[:])
            nc.sync.dma_start(out=sqa_s[:], in_=sqa_d[:])
            nc.sync.dma_start(out=cma_s[:], in_=cma_d[:])
            for g in range(N // GRP):
                gs = slice(g * GRP, (g + 1) * GRP)
                nc.sync.dma_start(out=xth_s[:, gs], in_=xth_d[:, gs])
                nc.sync.dma_start(out=xtl_s[:, gs], in_=xtl_d[:, gs])
            nc.sync.dma_start(out=cmb_s[:], in_=cmb_d[:])

            def emit_front(b):
                """mask + matmuls + sqrt-evacuation for block b."""
                bsl = slice(b, b + 1)
                blk = slice(b * 128, (b + 1) * 128)
                # maskshift = (comm_j != comm_i) * -SHIFT   (0 / -64, exact bf16)
                # (walrus rejects TensorScalarPtr on Pool, so this stays on
                # VectorE; bf16 runs at 4x there.)
                msk = mskp.tile([128, N], bf16, tag="msk")
                nc.vector.tensor_scalar(
                    out=msk[:],
                    in0=cmb_s[:],
                    scalar1=cma_s[:, bsl],
                    scalar2=-SHIFT,
                    op0=OP.not_equal,
                    op1=OP.mult,
                )

                dist = distp.tile([128, N], f32, tag="dist")
                for g in range(N // GRP):
                    ps = psp.tile([128, GRP], f32, tag="ps")
                    nsub = GRP // SUB
                    # weight-reuse order: (m2h x2 passes), m2l, ones
                    for s in range(nsub):
                        c0 = g * GRP + s * SUB
                        nc.tensor.matmul(
                            ps[:, s * SUB:(s + 1) * SUB],
                            m2h_s[:, blk], xth_s[:, c0:c0 + SUB],
                            start=True, stop=False,
                        )
                    for s in range(nsub):
                        c0 = g * GRP + s * SUB
                        nc.tensor.matmul(
                            ps[:, s * SUB:(s + 1) * SUB],
                            m2h_s[:, blk], xtl_s[:, c0:c0 + SUB],
                            start=False, stop=False,
                        )
                    for s in range(nsub):
                        c0 = g * GRP + s * SUB
                        nc.tensor.matmul(
                            ps[:, s * SUB:(s + 1) * SUB],
                            m2l_s[:, blk], xth_s[:, c0:c0 + SUB],
                            start=False, stop=False,
                        )
                    for s in range(nsub):
                        c0 = g * GRP + s * SUB
                        nc.tensor.matmul(
                            ps[:, s * SUB:(s + 1) * SUB],
                            one_s[:], sqr_s[:, c0:c0 + SUB],
                            start=False, stop=True,
                        )
                    # dist = sqrt(psum + sq_i)
                    nc.scalar.activation(
                        dist[:, g * GRP:(g + 1) * GRP],
                        ps[:],
                        AF.Sqrt,
                        bias=sqa_s[:, bsl],
                        scale=1.0,
                    )
                return dist, msk

            def emit_back(b, dist, msk):
                """md = dist + maskshift (in place), row-min, margin+sum."""
                bsl = slice(b, b + 1)
                mn = smallp.tile([128, 1], f32, tag="mn")
                nc.vector.tensor_tensor(
                    out=dist[:], in0=dist[:], in1=msk[:], op=OP.add
                )
                nc.vector.tensor_reduce(
                    out=mn[:], in_=dist[:], op=OP.min, axis=mybir.AxisListType.X
                )
                # bias = margin - minneg = MARGIN - (mn + SHIFT)
                cbias = smallp.tile([128, 1], f32, tag="cb")
                nc.vector.tensor_scalar(
                    out=cbias[:],
                    in0=mn[:],
                    scalar1=-1.0,
                    scalar2=MARGIN - SHIFT,
                    op0=OP.mult,
                    op1=OP.add,
                )
                # possum[:, b] = sum_j relu(md + bias)
                nc.scalar.activation(
                    msk[:],
                    dist[:],
                    AF.Relu,
                    bias=cbias[:],
                    scale=1.0,
                    accum_out=possum_s[:, bsl],
                )

            # software pipeline: block b's post-processing is emitted after
            # block b+1's matmuls/evacs, so PSUM evacuations never queue
            # behind the big ACT2 on the Scalar engine and the PE stays hot.
            pend = None
            for b in range(NBLK):
                front = emit_front(b)
                if pend is not None:
                    emit_back(b - 1, *pend)
                pend = front
            emit_back(NBLK - 1, *pend)

            nc.sync.dma_start(out=out_d[:], in_=possum_s[:])

    nc.compile()
    return nc


def get_nc():
    if "nc" not in _cache:
        _cache["nc"] = _build_nc()
    return _cache["nc"]


def _split_lo(v32):
    """v32 (f32) -> (hi, lo) bf16 arrays with hi + lo ~ v32 (2^-16 rel)."""
    h = v32.astype(BF16)
    lo = (v32 - h.astype(np.float32)).astype(BF16)
    return h, lo


def make_in_maps(embeddings, communities):
    X = np.ascontiguousarray(np.asarray(embeddings, dtype=np.float32))
    comm = np.asarray(communities).astype(np.int64)
    assert X.shape == (N, D) and comm.shape == (N,)

    sq64 = (X.astype(np.float64) ** 2).sum(axis=1)
    sq = sq64.astype(np.float32)
    sqa_full = (sq64 + DIAG_EPS).astype(np.float32)
    commf = comm.astype(np.float32)

    xt = np.ascontiguousarray(X.T)                       # [128, 8192] f32
    xth, xtl = _split_lo(xt)
    # sq -> 3-way bf16 split (exact to ~2^-24 rel)
    sqh = sq.astype(BF16)
    r = sq - sqh.astype(np.float32)
    sqm = r.astype(BF16)
    sql = (r - sqm.astype(np.float32)).astype(BF16)
    sqr = np.ascontiguousarray(np.stack([sqh, sqm, sql], axis=0))  # [3, N] bf16
    one = np.ones((3, D), dtype=BF16)
    cmb = np.ascontiguousarray(
        np.broadcast_to(commf[None, :], (128, N))
    ).astype(BF16)

    in_maps = []
    for c in range(NCORES):
        rows = slice(c * RPC, (c + 1) * RPC)
        m2 = np.ascontiguousarray((-2.0 * X[rows]).T)    # [128, 1024] f32
        m2h, m2l = _split_lo(m2)
        sqa = np.ascontiguousarray(sqa_full[rows].reshape(NBLK, 128).T)
        cma = np.ascontiguousarray(commf[rows].reshape(NBLK, 128).T)
        in_maps.append(
            dict(xth=np.ascontiguousarray(xth), xtl=np.ascontiguousarray(xtl),
                 m2h=np.ascontiguousarray(m2h), m2l=np.ascontiguousarray(m2l),
                 sqr=sqr, one=one, sqa=sqa, cmb=cmb, cma=cma)
        )
    return in_maps, comm


def finalize(results, comm):
    """results: list (per core) of dicts with 'possum' [128, NBLK] f32."""
    total = 0.0
    for r in results:
        total += float(r["possum"].astype(np.float64).sum())
    counts = np.bincount(comm)
    counts = counts[counts < N]  # rows with no negative are invalid
    cnt = int((counts * (counts - 1)).sum())
    if cnt == 0:
        return np.array(0.0, dtype=np.float32)
    loss = np.float32(total) / np.float32(cnt)
    return np.array(loss, dtype=np.float32)


def kernel(embeddings, communities):
    from concourse.bass_utils import run_bass_kernel_spmd

    nc = get_nc()
    in_maps, comm = make_in_maps(embeddings, communities)
    res = run_bass_kernel_spmd(nc, in_maps, core_ids=list(range(NCORES)))
    return finalize(res.results, comm)



# revision 8
# speedup vs baseline: 1.2536x; 1.2536x over previous
"""Trainium2 Bass kernel for CommunityPreservationLoss (triplet margin loss
over pairwise distances with hardest-negative mining).

Strategy (8 NeuronCores, SPMD), v2 — community-sorted windowing:
  - Host sorts nodes by community id. Positives (same community) become
    contiguous column runs. Each core's 8192 columns are ROTATED by
    (1024*c - 192) so that for anchor block b (128 sorted rows), every
    positive lies in the fixed local window [128b, 128b+512). Window
    offsets are therefore compile-time constants, identical on all cores;
    all per-core variation lives in the input data.
  - PE does a SINGLE bf16 matmul pass per block: psum u = (-2 X_blk)^T X
    (the baseline needed 4 passes: bf16 hi/lo split + sq broadcast).
  - Hardest-negative min runs in the u+sq_j domain (sqrt is monotonic, so
    sqrt only on the [128,1] winner): DVE tensor_tensor_reduce fuses
    (psum + sq_j_f16) -> row-min straight out of PSUM, one elementwise
    pass instead of the baseline's three. The window columns are excluded
    from the plain pieces and min'd separately with a host-built f16 mask
    mskq = sq_j + BIG*(same community).
  - GpSimd offloads group 3: add sq_j then two min-folds 2048->512, DVE
    finishes with a [128,512] reduce. GpSimd also adds sq_j on the window
    (sqw, which carries +1.0 on the diagonal element so sqrt never sees a
    negative d2_ii) and applies the -64 different-community shift before
    the margin relu.
  - ACT only touches the 512-wide window: dist_w = Sqrt(tmpw + sq_i),
    then Relu(dist_w - 64*diff + (margin - minneg)) with a fused row-sum
    (accum_out) into possum. Same-community pairs contribute the triplet
    margin; the diagonal (dist ~ 1, minneg > 9) and shifted
    different-community pairs relu to zero.
  - Host sums the 8x[128,8] partials (f64) and divides by the exact
    triplet count from a bincount of communities.
"""

import os
import numpy as np
import ml_dtypes

BF16 = ml_dtypes.bfloat16
USE_F16 = os.environ.get("K_NO_F16", "") == ""
USE_TTR = os.environ.get("K_NO_TTR", "") == ""
F16 = np.float16 if USE_F16 else BF16

N = 8192           # nodes
D = 128            # embedding dim
NCORES = 8
RPC = N // NCORES  # rows per core = 1024
NBLK = RPC // 128  # anchor blocks per core = 8
GRP = 2048         # psum tile width (4 banks)
SUB = 512          # matmul moving width
NGRP = N // GRP
WIN = 512          # positive window width
MARGIN = 1.0
SHIFT = 64.0       # additive mask shift for different-community pairs
BIG = 30000.0      # same-community shift excluding positives from the min
EPS_DIAG = 1.0 if USE_F16 else 4.0  # keeps d2_ii positive under sq rounding
MAXCOMM = 193      # window-coverage guarantee: community size must be <= this
INIT_MIN = 1.0e30

_cache = {}


def _build_nc():
    import concourse.tile as tile
    from concourse import bacc, mybir

    f32 = mybir.dt.float32
    f16 = mybir.dt.float16 if USE_F16 else mybir.dt.bfloat16
    bf16 = mybir.dt.bfloat16
    AF = mybir.ActivationFunctionType
    OP = mybir.AluOpType

    nc = bacc.Bacc("TRN2", target_bir_lowering=False, debug=False)

    m2_d = nc.dram_tensor("m2", [D, RPC], bf16, kind="ExternalInput").ap()
    xt_d = nc.dram_tensor("xt", [D, N], bf16, kind="ExternalInput").ap()
    sqb_d = nc.dram_tensor("sqb", [128, N], f16, kind="ExternalInput").ap()
    mskq_d = nc.dram_tensor("mskq", [128, NBLK * WIN], f16, kind="ExternalInput").ap()
    sqw_d = nc.dram_tensor("sqw", [128, NBLK * WIN], f16, kind="ExternalInput").ap()
    mkb_d = nc.dram_tensor("mkb", [128, NBLK * WIN], bf16, kind="ExternalInput").ap()
    sqa_d = nc.dram_tensor("sqa", [128, NBLK], f32, kind="ExternalInput").ap()
    out_d = nc.dram_tensor("possum", [128, NBLK], f32, kind="ExternalOutput").ap()

    with tile.TileContext(nc) as tc:
        with (
            tc.tile_pool(name="const", bufs=1) as constp,
            tc.tile_pool(name="scr", bufs=2) as scrp,
            tc.tile_pool(name="win", bufs=2) as winp,
            tc.tile_pool(name="small", bufs=4) as smallp,
            tc.tile_pool(name="ps", bufs=2, space="PSUM") as psp,
        ):
            m2_s = constp.tile([D, RPC], bf16, tag="m2")
            xt_s = constp.tile([D, N], bf16, tag="xt")
            sqb_s = constp.tile([128, N], f16, tag="sqb")
            mskq_s = constp.tile([128, NBLK * WIN], f16, tag="mskq")
            sqw_s = constp.tile([128, NBLK * WIN], f16, tag="sqw")
            mkb_s = constp.tile([128, NBLK * WIN], bf16, tag="mkb")
            sqa_s = constp.tile([128, NBLK], f32, tag="sqa")
            possum_s = constp.tile([128, NBLK], f32, tag="possum")

            # loads in first-use order so block-0 compute starts ASAP
            nc.sync.dma_start(out=m2_s[:], in_=m2_d[:])
            nc.sync.dma_start(out=sqa_s[:], in_=sqa_d[:])
            nc.sync.dma_start(out=xt_s[:, 0:GRP], in_=xt_d[:, 0:GRP])
            nc.sync.dma_start(out=sqb_s[:, 0:GRP], in_=sqb_d[:, 0:GRP])
            nc.sync.dma_start(out=mskq_s[:], in_=mskq_d[:])
            nc.sync.dma_start(out=sqw_s[:], in_=sqw_d[:])
            for g in range(1, NGRP):
                gs = slice(g * GRP, (g + 1) * GRP)
                nc.sync.dma_start(out=xt_s[:, gs], in_=xt_d[:, gs])
                nc.sync.dma_start(out=sqb_s[:, gs], in_=sqb_d[:, gs])
            nc.sync.dma_start(out=mkb_s[:], in_=mkb_d[:])


            def min_into(out_col, in0, in1, scr_ap):
                """row-min of (in0 + in1) into mins column, fused if enabled."""
                if USE_TTR:
                    nc.vector.tensor_tensor_reduce(
                        out=scr_ap, in0=in0, in1=in1,
                        scale=1.0, scalar=INIT_MIN,
                        op0=OP.add, op1=OP.min, accum_out=out_col,
                    )
                else:
                    nc.vector.tensor_tensor(out=scr_ap, in0=in0, in1=in1, op=OP.add)
                    nc.vector.tensor_reduce(
                        out=out_col, in_=scr_ap, op=OP.min,
                        axis=mybir.AxisListType.X,
                    )

            def emit_front(b):
                blk = slice(b * 128, (b + 1) * 128)
                bsl = slice(b, b + 1)
                w0, w1 = 128 * b, 128 * b + WIN
                wloc = slice(b * WIN, (b + 1) * WIN)
                # mins cols: 0=win 1=pieceB 2=g1 3=g2 4=g3 5=pieceA(b>0)
                mins = smallp.tile([128, 8], f32, tag="mins")
                distw = winp.tile([128, WIN], f32, tag="distw")
                for g in range(NGRP):
                    ps = psp.tile([128, GRP], f32, tag="ps")
                    for s in range(GRP // SUB):
                        c0 = g * GRP + s * SUB
                        nc.tensor.matmul(
                            ps[:, s * SUB:(s + 1) * SUB],
                            m2_s[:, blk], xt_s[:, c0:c0 + SUB],
                            start=True, stop=True,
                        )
                    gsl = slice(g * GRP, (g + 1) * GRP)
                    if g == 0:
                        # window: d2 = u + sq_j(+eps on diag) + sq_i, dist = sqrt
                        # (GpSimd cannot read PSUM, so this add is on DVE)
                        tmpw = winp.tile([128, WIN], f32, tag="tmpw")
                        nc.vector.tensor_tensor(
                            out=tmpw[:], in0=ps[:, w0:w1], in1=sqw_s[:, wloc],
                            op=OP.add,
                        )
                        nc.scalar.activation(
                            distw[:], tmpw[:], AF.Sqrt,
                            bias=sqa_s[:, bsl], scale=1.0,
                        )
                        # masked window min (same-community pushed up by BIG)
                        wscr = winp.tile([128, WIN], f32, tag="wscr")
                        min_into(mins[:, 0:1], ps[:, w0:w1], mskq_s[:, wloc], wscr[:])
                        # plain pieces around the window
                        scr = scrp.tile([128, GRP], f32, tag="tscr")
                        wb = GRP - w1
                        min_into(mins[:, 1:2], ps[:, w1:GRP], sqb_s[:, w1:GRP], scr[:, 0:wb])
                        if b > 0:
                            min_into(mins[:, 5:6], ps[:, 0:w0], sqb_s[:, 0:w0], scr[:, wb:wb + w0])
                    else:  # g in (1, 2, 3): fused (psum + sq_j) -> row-min
                        scr = scrp.tile([128, GRP], f32, tag="tscr")
                        min_into(mins[:, g + 1:g + 2], ps[:], sqb_s[:, gsl], scr[:])
                return mins, distw

            def emit_back(b, mins, distw):
                bsl = slice(b, b + 1)
                wloc = slice(b * WIN, (b + 1) * WIN)
                ncand = 6 if b > 0 else 5
                mu = smallp.tile([128, 1], f32, tag="mu")
                nc.vector.tensor_reduce(
                    out=mu[:], in_=mins[:, 0:ncand],
                    op=OP.min, axis=mybir.AxisListType.X,
                )
                mnd = smallp.tile([128, 1], f32, tag="mnd")
                nc.scalar.activation(
                    mnd[:], mu[:], AF.Sqrt, bias=sqa_s[:, bsl], scale=1.0
                )
                cbias = smallp.tile([128, 1], f32, tag="cb")
                nc.vector.tensor_scalar(
                    out=cbias[:], in0=mnd[:],
                    scalar1=-1.0, scalar2=MARGIN, op0=OP.mult, op1=OP.add,
                )
                # mdw = dist + (-SHIFT on different-community), in place
                nc.vector.tensor_tensor(
                    out=distw[:], in0=distw[:], in1=mkb_s[:, wloc], op=OP.add
                )
                ascr = winp.tile([128, WIN], f32, tag="ascr")
                nc.scalar.activation(
                    ascr[:], distw[:], AF.Relu,
                    bias=cbias[:], scale=1.0, accum_out=possum_s[:, bsl],
                )

            pend = None
            for b in range(NBLK):
                front = emit_front(b)
                if pend is not None:
                    emit_back(b - 1, *pend)
                pend = front
            emit_back(NBLK - 1, *pend)

            nc.sync.dma_start(out=out_d[:], in_=possum_s[:])

    nc.compile()
    return nc


def get_nc():
    if "nc" not in _cache:
        _cache["nc"] = _build_nc()
    return _cache["nc"]


def make_in_maps(embeddings, communities):
    X = np.ascontiguousarray(np.asarray(embeddings, dtype=np.float32))
    comm = np.asarray(communities).astype(np.int64)
    assert X.shape == (N, D) and comm.shape == (N,)

    order = np.argsort(comm, kind="stable")
    Xs = X[order]
    cs = comm[order]
    sqs = (Xs.astype(np.float64) ** 2).sum(axis=1).astype(np.float32)

    xtq = np.ascontiguousarray(Xs.T).astype(BF16)        # [128, N]
    m2q = np.ascontiguousarray((-2.0 * Xs).T).astype(BF16)
    jw = np.arange(WIN)
    pp = np.arange(128)
    diag = (jw[None, :] == (192 + pp)[:, None])          # [128, WIN]

    in_maps = []
    for c in range(NCORES):
        rows = slice(c * RPC, (c + 1) * RPC)
        rot = (RPC * c - 192) % N
        colidx = (np.arange(N) + rot) % N
        xt = np.ascontiguousarray(xtq[:, colidx])
        sq_loc = sqs[colidx]
        cl = cs[colidx]
        sqb = np.ascontiguousarray(
            np.broadcast_to(sq_loc[None, :], (128, N))
        ).astype(F16)
        m2 = np.ascontiguousarray(m2q[:, rows])
        sqa = np.ascontiguousarray(sqs[rows].reshape(NBLK, 128).T)

        # per-block windows: local cols [128b, 128b+WIN)
        wcol = (128 * np.arange(NBLK))[:, None] + jw[None, :]     # [NBLK, WIN]
        wsq = sq_loc[wcol]                                        # [NBLK, WIN]
        wcl = cl[wcol]
        ca = cs[rows].reshape(NBLK, 128)                          # [NBLK, 128]
        same = wcl[:, None, :] == ca[:, :, None]                  # [NBLK, 128, WIN]
        mskq = (wsq[:, None, :] + BIG * same).astype(F16)
        sqw = (wsq[:, None, :] + EPS_DIAG * diag[None, :, :]).astype(F16)
        mkb = ((-SHIFT) * (~same)).astype(BF16)
        # [NBLK, 128, WIN] -> [128, NBLK*WIN]
        mskq = np.ascontiguousarray(mskq.transpose(1, 0, 2).reshape(128, NBLK * WIN))
        sqw = np.ascontiguousarray(sqw.transpose(1, 0, 2).reshape(128, NBLK * WIN))
        mkb = np.ascontiguousarray(mkb.transpose(1, 0, 2).reshape(128, NBLK * WIN))

        in_maps.append(
            dict(m2=m2, xt=xt, sqb=sqb, mskq=mskq, sqw=sqw, mkb=mkb, sqa=sqa)
        )
    return in_maps, comm


def finalize(results, comm):
    total = 0.0
    for r in results:
        total += float(r["possum"].astype(np.float64).sum())
    counts = np.bincount(comm)
    counts = counts[counts < N]  # rows with no negative are invalid
    cnt = int((counts * (counts - 1)).sum())
    if cnt == 0:
        return np.array(0.0, dtype=np.float32)
    loss = np.float32(total) / np.float32(cnt)
    return np.array(loss, dtype=np.float32)


def _numpy_fallback(embeddings, communities):
    X = np.asarray(embeddings, dtype=np.float64)
    comm = np.asarray(communities).astype(np.int64)
    sq = (X ** 2).sum(1)
    d2 = np.maximum(sq[:, None] + sq[None, :] - 2.0 * (X @ X.T), 0.0)
    dist = np.sqrt(d2)
    same = comm[:, None] == comm[None, :]
    neg = ~same
    mn = np.where(neg, dist, np.inf).min(1)
    has = neg.any(1)
    pos = same & ~np.eye(len(comm), dtype=bool) & has[:, None]
    tl = np.maximum(dist - mn[:, None] + MARGIN, 0.0)
    cnt = pos.sum()
    if cnt == 0:
        return np.array(0.0, dtype=np.float32)
    return np.array(np.where(pos, tl, 0.0).sum() / cnt, dtype=np.float32)


def kernel(embeddings, communities):
    comm_arr = np.asarray(communities).astype(np.int64)
    if np.bincount(comm_arr).max() > MAXCOMM:
        # window-coverage guarantee violated (never for the target regime)
        return _numpy_fallback(embeddings, communities)

    from concourse.bass_utils import run_bass_kernel_spmd

    nc = get_nc()
    in_maps, comm = make_in_maps(embeddings, communities)
    res = run_bass_kernel_spmd(nc, in_maps, core_ids=list(range(NCORES)))
    return finalize(res.results, comm)
